# revision 38
# baseline (speedup 1.0000x reference)
"""GAU (relu^2 gated attention unit) kernel for 8 TRN2 NeuronCores.

Problem: B=4, T=2048, H=768 dense GAU block:
  x = LN(hidden); U,Q,K,V = silu(x @ W*^T + b*)
  sim = Q K^T / sqrt(T); A = relu(sim)^2; out = (U * (A V)) @ Wo^T + bo

Sharding: 8 cores = 4 batches x 2 sequence halves. Each core owns 1024
query rows; it recomputes K/V for its batch's full 2048 rows (cheaper
than cross-core collectives at this size). Inputs are passed per-core
with the OWN half first so all cores run the identical program
(relu^2 attention is permutation-invariant along the key axis).

Host-side weight preprocessing (weight-only algebra, no activations):
  - ln_gamma folded into W{u,q,k,v}; ln_beta+bias folded into per-proj bias
  - 1/sqrt(T)^2 = 1/T folded into Wo (relu(s*x)^2 = s^2 relu(x)^2)

On-device layouts (all matmuls fp32r = TF32, 1 cyc/row on the PE):
  - x^T [H, T] with H on partitions (6 k-subtiles) -> LN stats via
    ones-vector matmuls; normalize with per-column r/p broadcast tiles.
  - Q^T/K^T/U^T produced as [d, t] (silu+bias = one ACT op, bias on
    partitions); V natural [t, d] (lhsT of the AV matmul).
  - sim^T [j, i] = (K^T[:,j-tile]).T @ Q^T, so A^T[j, i] feeds the AV
    matmul directly: AV^T[d, i] = (V[j, d-tile]).T @ A^T. No transposes.
  - gating G = U^T * AV^T written in place into U^T; final projection
    contracts d with Wo^T [d, h] -> natural [t, h] output rows.
"""

import os
import numpy as np

import concourse.tile as tile
from concourse import bacc, mybir
from concourse.bass_utils import run_bass_kernel_spmd

F32 = mybir.dt.float32
F32R = mybir.dt.float32r

P = 128
H = 768
KS = H // P            # 6 k-subtiles over the H contraction
B = 4
T = 2048
TO = 1024              # own query rows per core
DT = H // P            # 6 d-tiles
CA = 512               # phase-A chunk (LN + Q/U)
NCA = T // CA          # 4
CC = 256               # phase-C chunk (K/V projections)
NCC = T // CC          # 8
NJ = T // P            # 16 key tiles
IH = 512               # i-half size for attention PSUM capacity
NIH = TO // IH         # 2
EPS = 1e-5

AF = mybir.ActivationFunctionType
ALU = mybir.AluOpType

_prog_cache = {}


def _build_program():
    nc = bacc.Bacc()
    xT = nc.declare_dram_parameter("xT", [H, T], F32R, isOutput=False)
    wq = nc.declare_dram_parameter("wq", [H, H], F32R, isOutput=False)
    wu = nc.declare_dram_parameter("wu", [H, H], F32R, isOutput=False)
    wk = nc.declare_dram_parameter("wk", [H, H], F32R, isOutput=False)
    wv = nc.declare_dram_parameter("wv", [H, H], F32R, isOutput=False)
    wo = nc.declare_dram_parameter("wo", [H, H], F32R, isOutput=False)
    bq = nc.declare_dram_parameter("bq", [H], F32, isOutput=False)
    bu = nc.declare_dram_parameter("bu", [H], F32, isOutput=False)
    bk = nc.declare_dram_parameter("bk", [H], F32, isOutput=False)
    bv = nc.declare_dram_parameter("bv", [H], F32, isOutput=False)
    bo = nc.declare_dram_parameter("bo", [H], F32, isOutput=False)
    out = nc.declare_dram_parameter("out", [TO, H], F32, isOutput=True)

    with tile.TileContext(nc) as tc:
        _emit(nc, tc, xT, (wq, wu, wk, wv, wo), (bq, bu, bk, bv, bo), out)
    nc.finalize()
    return nc


def _emit(nc, tc, xT, ws, bs, out):
    wq_d, wu_d, wk_d, wv_d, wo_d = ws
    bq_d, bu_d, bk_d, bv_d, bo_d = bs

    from contextlib import ExitStack

    with ExitStack() as ctx:
        const = ctx.enter_context(tc.tile_pool(name="const", bufs=1))
        big = ctx.enter_context(tc.tile_pool(name="big", bufs=1))
        wpool = ctx.enter_context(tc.tile_pool(name="wpool", bufs=2))
        dram = ctx.enter_context(tc.tile_pool(name="dram", bufs=1, space="DRAM"))

        # ---- constants (memsets only; bias DMAs are issued later so they
        # don't queue ahead of the critical first x-chunk loads) ----
        ones_f = const.tile([P, 1], F32, name="ones_f")
        nc.vector.memset(ones_f, 1.0)
        ones = const.tile([P, 1], F32R, name="ones")
        nc.vector.tensor_copy(ones, ones_f)
        ones_rf = const.tile([1, P], F32, name="ones_rf")
        nc.vector.memset(ones_rf, 1.0)
        ones_r = const.tile([1, P], F32R, name="ones_r")
        nc.vector.tensor_copy(ones_r, ones_rf)

        def load_biases():
            # per-partition biases for d-on-partition layouts: [128, 6]
            bq_t = const.tile([P, DT], F32, name="bq_t")
            nc.sync.dma_start(
                out=bq_t, in_=bq_d[:].rearrange("(dt p) -> p dt", p=P))
            bu_t = const.tile([P, DT], F32, name="bu_t")
            nc.sync.dma_start(
                out=bu_t, in_=bu_d[:].rearrange("(dt p) -> p dt", p=P))
            bk_t = const.tile([P, DT], F32, name="bk_t")
            nc.sync.dma_start(
                out=bk_t, in_=bk_d[:].rearrange("(dt p) -> p dt", p=P))
            # free-dim (replicated) biases for t-on-partition layouts
            bv_rep = const.tile([P, H], F32, name="bv_rep")
            nc.gpsimd.dma_start(
                out=bv_rep, in_=bv_d[:][None, :].to_broadcast([P, H]))
            bo_rep = const.tile([P, H], F32, name="bo_rep")
            nc.gpsimd.dma_start(
                out=bo_rep, in_=bo_d[:][None, :].to_broadcast([P, H]))
            return bq_t, bu_t, bk_t, bv_rep, bo_rep

        # ---- persistent activations (KT/VN created later, after phase A
        # pools release, so their SBUF space materializes late) ----
        QT = big.tile([P, DT, TO], F32R, name="QT")    # Q^T [d, t_own]
        UT = big.tile([P, DT, TO], F32R, name="UT")    # U^T; becomes G in place
        xhat_ch_dram = [
            dram.tile([P, KS, CA], F32R, name=f"xhat{c}") for c in range(NCA)]

        def load_w(pool, src_d, name):
            w = pool.tile([P, KS, H], F32R, tag="w", name=name)
            nc.sync.dma_start(
                out=w, in_=src_d[:].rearrange("(ks p) d -> p ks d", p=P))
            return w

        # =========== Phase A: LN stats + normalize (A0), Q/U (A1) ==========
        # ACT discipline: phase A uses only Sqrt / Square / Copy on the
        # scalar engine. Square and Copy are in every activation table set,
        # so the table never thrashes between the sqrt and silu sets. The
        # silu+bias for Q/U is applied as a batched pass at the end.
        pctx = ctx.enter_context(ExitStack())
        ps_pr = pctx.enter_context(
            tc.tile_pool(name="ps_pr", bufs=4, space="PSUM"))
        xs_pool = ctx.enter_context(tc.tile_pool(name="xs", bufs=2))
        xs_tiles = []
        wpA_ctx = ctx.enter_context(ExitStack())
        wpA = wpA_ctx.enter_context(tc.tile_pool(name="wpA", bufs=2))
        with ExitStack() as actx:
            xh = actx.enter_context(tc.tile_pool(name="xh", bufs=4))
            st = actx.enter_context(tc.tile_pool(name="st", bufs=2))
            ps_st = actx.enter_context(
                tc.tile_pool(name="ps_st", bufs=1, space="PSUM"))
            ps_bc = actx.enter_context(
                tc.tile_pool(name="ps_bc", bufs=1, space="PSUM"))

            # DMA issue order = need order: x chunks gate all early PE work;
            # each weight lands just before its first matmul.
            xT_r = xT[:].rearrange("(ks p) t -> p ks t", p=P)
            xh_ch = [xh.tile([P, KS, CA], F32R, tag="xh", bufs=4,
                             name=f"xc{ch}") for ch in range(NCA)]

            def load_xc(ch):
                for k in range(KS):
                    nc.sync.dma_start(
                        out=xh_ch[ch][:, k, :],
                        in_=xT_r[:, k, ch * CA:(ch + 1) * CA])

            load_xc(0)
            wq_t = load_w(wpA, wq_d, "wq_t")
            load_xc(1)
            wu_t = load_w(wpA, wu_d, "wu_t")
            load_xc(2)
            load_xc(3)
            bq_t, bu_t, bk_t, bv_rep, bo_rep = load_biases()
            wk_t = load_w(wpool, wk_d, "wk_t")
            wv_t = load_w(wpool, wv_d, "wv_t")

            # ---- A0: stats + normalize + spill ----
            for ch in range(NCA):
                sl = slice(ch * CA, (ch + 1) * CA)
                xc = xh_ch[ch]
                ps_s = ps_st.tile([1, CA], F32, tag="s", name=f"ps_s{ch}")
                ps_q = ps_st.tile([1, CA], F32, tag="ss", name=f"ps_q{ch}")
                for k in range(KS):
                    nc.tensor.matmul(ps_s, ones, xc[:, k, :],
                                     start=(k == 0), stop=(k == KS - 1))
                for k in range(KS):
                    x2 = st.tile([P, CA], F32R, tag="x2", name=f"x2_{ch}_{k}")
                    nc.scalar.activation(
                        out=x2, in_=xc[:, k, :], func=AF.Square)
                    nc.tensor.matmul(ps_q, ones, x2,
                                     start=(k == 0), stop=(k == KS - 1))
                s1 = st.tile([1, CA], F32, tag="s1", name=f"s1_{ch}", bufs=1)
                nc.vector.tensor_copy(s1, ps_s)
                q1 = st.tile([1, CA], F32, tag="q1", name=f"q1_{ch}", bufs=1)
                nc.vector.tensor_copy(q1, ps_q)
                # var = ss/H - (s/H)^2 ; r = rsqrt(var+eps); p = -(s/H)*r
                t1 = st.tile([1, CA], F32, tag="t1", name=f"t1_{ch}", bufs=1)
                nc.vector.scalar_tensor_tensor(
                    t1, s1, 1.0 / (H * H), s1, ALU.mult, ALU.mult)
                nc.vector.scalar_tensor_tensor(
                    q1, q1, 1.0 / H, t1, ALU.mult, ALU.subtract)
                nc.vector.tensor_scalar_add(q1, q1, EPS)
                nc.vector.reciprocal(t1, q1)
                r1 = st.tile([1, CA], F32R, tag="r1", name=f"r1_{ch}", bufs=1)
                nc.scalar.activation(out=r1, in_=t1, func=AF.Sqrt)
                p1 = st.tile([1, CA], F32R, tag="p1", name=f"p1_{ch}", bufs=1)
                nc.vector.scalar_tensor_tensor(
                    p1, s1, -1.0 / H, r1, ALU.mult, ALU.mult)
                # replicate r/p across partitions with a K=1 matmul
                rr = ps_bc.tile([P, CA], F32, tag="rr", name=f"rr{ch}")
                nc.tensor.matmul(rr, ones_r, r1, start=True, stop=True)
                pp = ps_bc.tile([P, CA], F32, tag="pp", name=f"pp{ch}")
                nc.tensor.matmul(pp, ones_r, p1, start=True, stop=True)
                for k in range(KS):
                    nc.vector.tensor_mul(xc[:, k, :], xc[:, k, :], rr)
                    nc.vector.tensor_add(xc[:, k, :], xc[:, k, :], pp)
                nc.sync.dma_start(out=xhat_ch_dram[ch], in_=xc)
                # pre-issue the phase-C reload DMAs for this chunk now, so
                # they sit ahead of later spills in the DMA queues
                for half in range(2):
                    xs = xs_pool.tile([P, KS, CC], F32R, tag="xs",
                                      name=f"xs{2 * ch + half}")
                    nc.sync.dma_start(
                        out=xs,
                        in_=xhat_ch_dram[ch][:, :, half * CC:(half + 1) * CC])
                    xs_tiles.append(xs)

            # ---- A1: Q/U projections from the own half. Copy-evict, then
            # silu+bias per d-tile as soon as both chunks are in (keeps the
            # sqrt/silu table sets from thrashing AND spreads the silu work
            # over A1's ACT slack instead of phase C's) ----
            def qu_mm(w_t, dst, ch, dt):
                sl = slice(ch * CA, (ch + 1) * CA)
                psum = ps_pr.tile([P, CA], F32, tag="pp512",
                                  name=f"pj{ch}_{dt}")
                for k in range(KS):
                    nc.tensor.matmul(
                        psum, w_t[:, k, dt * P:(dt + 1) * P],
                        xh_ch[ch][:, k, :],
                        start=(k == 0), stop=(k == KS - 1))
                nc.scalar.activation(out=dst[:, dt, sl], in_=psum, func=AF.Copy)

            for w_t, b_t, dst in ((wq_t, bq_t, QT), (wu_t, bu_t, UT)):
                for dt in range(DT):
                    qu_mm(w_t, dst, 0, dt)
                    qu_mm(w_t, dst, 1, dt)
                    nc.scalar.activation(
                        out=dst[:, dt, :], in_=dst[:, dt, :], func=AF.Silu,
                        bias=b_t[:, dt:dt + 1])
        wpA_ctx.close()

        # ========= Phase C: K/V projections (stream x^ back) =========
        kv = ctx.enter_context(tc.tile_pool(name="kv", bufs=1))
        KT = kv.tile([P, DT, T], F32R, name="KT")      # K^T [d, t_all]
        VN = kv.tile([P, NJ, H], F32R, name="VN")      # V natural [t_all, d]
        if True:
            for ch in range(NCC):
                sl = slice(ch * CC, (ch + 1) * CC)
                xs = xs_tiles[ch]
                # K^T [d, t]
                for dt in range(DT):
                    psum = ps_pr.tile([P, CC], F32, tag="pp512",
                                      name=f"pk{ch}_{dt}")
                    for k in range(KS):
                        nc.tensor.matmul(
                            psum, wk_t[:, k, dt * P:(dt + 1) * P], xs[:, k, :],
                            start=(k == 0), stop=(k == KS - 1))
                    nc.scalar.activation(
                        out=KT[:, dt, sl], in_=psum, func=AF.Silu,
                        bias=bk_t[:, dt:dt + 1])
                # V natural [t, d] (bias varies along free dim -> TT add + silu)
                for ts in range(CC // P):
                    tj = ch * (CC // P) + ts  # global t-subtile
                    for nh in range(2):
                        hsl = slice(nh * 384, (nh + 1) * 384)
                        psum = ps_pr.tile([P, 384], F32, tag="pp512",
                                          name=f"pv{ch}_{ts}_{nh}")
                        for k in range(KS):
                            nc.tensor.matmul(
                                psum, xs[:, k, ts * P:(ts + 1) * P],
                                wv_t[:, k, hsl],
                                start=(k == 0), stop=(k == KS - 1))
                        nc.vector.tensor_add(VN[:, tj, hsl], psum, bv_rep[:, hsl])
                    nc.scalar.activation(
                        out=VN[:, tj, :], in_=VN[:, tj, :], func=AF.Silu)

        pctx.close()

        # ================= Attention: sim^T -> relu^2 -> AV =================
        wo_t = load_w(wpool, wo_d, "wo_t")
        with ExitStack() as bctx:
            at_pool = bctx.enter_context(tc.tile_pool(name="at", bufs=2))
            ob_pool = bctx.enter_context(tc.tile_pool(name="ob", bufs=3))
            ps_sim = bctx.enter_context(
                tc.tile_pool(name="ps_sim", bufs=2, space="PSUM"))
            ps_av = bctx.enter_context(
                tc.tile_pool(name="ps_av", bufs=1, space="PSUM"))

            def out_proj(ih):
                # output projection for one i-half; psum reuses av tags so it
                # overlaps the other half's attention without extra banks
                for tt in range(ih * (IH // P), (ih + 1) * (IH // P)):
                    for nh in range(2):
                        hsl = slice(nh * 384, (nh + 1) * 384)
                        psum = ps_av.tile([P, IH], F32, tag=f"av{nh}",
                                          name=f"po{tt}_{nh}")[:, :384]
                        for dt in range(DT):
                            nc.tensor.matmul(
                                psum, UT[:, dt, tt * P:(tt + 1) * P],
                                wo_t[:, dt, hsl],
                                start=(dt == 0), stop=(dt == DT - 1))
                        osb = ob_pool.tile([P, 384], F32, tag="osb",
                                           name=f"osb{tt}_{nh}")
                        nc.vector.tensor_add(osb, psum, bo_rep[:, hsl])
                        nc.sync.dma_start(
                            out=out[:][tt * P:(tt + 1) * P, hsl], in_=osb)

            for ih in range(NIH):
                isl = slice(ih * IH, (ih + 1) * IH)
                ps_avs = [ps_av.tile([P, IH], F32, tag=f"av{dt}",
                                     name=f"av{ih}_{dt}") for dt in range(DT)]
                for jt in range(NJ):
                    psum = ps_sim.tile([P, IH], F32, tag="sim", name=f"sim{ih}_{jt}")
                    for k in range(KS):
                        nc.tensor.matmul(
                            psum, KT[:, k, jt * P:(jt + 1) * P], QT[:, k, isl],
                            start=(k == 0), stop=(k == KS - 1))
                    at = at_pool.tile([P, IH], F32R, tag="at", name=f"at{ih}_{jt}")
                    nc.scalar.activation(out=at, in_=psum, func=AF.Relu)
                    nc.vector.tensor_mul(at, at, at)
                    for dt in range(DT):
                        nc.tensor.matmul(
                            ps_avs[dt], VN[:, jt, dt * P:(dt + 1) * P], at,
                            start=(jt == 0), stop=(jt == NJ - 1))
                # gating: G = U^T * AV^T, in place into UT
                for dt in range(DT):
                    nc.vector.tensor_mul(
                        UT[:, dt, isl], ps_avs[dt], UT[:, dt, isl])
                out_proj(ih)


def kernel(**inputs):
    hs = np.ascontiguousarray(np.asarray(inputs["hidden_states"], dtype=np.float32))
    gamma = np.asarray(inputs["ln_gamma"], dtype=np.float32)
    beta = np.asarray(inputs["ln_beta"], dtype=np.float32)

    def prep_w(w):
        w = np.asarray(w, dtype=np.float32)
        return np.ascontiguousarray((w * gamma[None, :]).T)

    def prep_b(w, b):
        w = np.asarray(w, dtype=np.float32)
        b = np.asarray(b, dtype=np.float32)
        return (w @ beta + b).astype(np.float32)

    wq = prep_w(inputs["Wq_w"]);  bq = prep_b(inputs["Wq_w"], inputs["Wq_b"])
    wu = prep_w(inputs["Wu_w"]);  bu = prep_b(inputs["Wu_w"], inputs["Wu_b"])
    wk = prep_w(inputs["Wk_w"]);  bk = prep_b(inputs["Wk_w"], inputs["Wk_b"])
    wv = prep_w(inputs["Wv_w"]);  bv = prep_b(inputs["Wv_w"], inputs["Wv_b"])
    wo = np.ascontiguousarray(np.asarray(inputs["Wo_w"], np.float32).T) / np.float32(T)
    bo = np.asarray(inputs["Wo_b"], dtype=np.float32)

    if "nc" not in _prog_cache:
        _prog_cache["nc"] = _build_program()
    nc = _prog_cache["nc"]

    in_maps = []
    for c in range(8):
        b, h = divmod(c, 2)
        own = hs[b, h * TO:(h + 1) * TO]
        oth = hs[b, (1 - h) * TO:(2 - h) * TO]
        xT = np.ascontiguousarray(np.concatenate([own, oth], axis=0).T)
        in_maps.append({
            "xT": xT, "wq": wq, "wu": wu, "wk": wk, "wv": wv, "wo": wo,
            "bq": bq, "bu": bu, "bk": bk, "bv": bv, "bo": bo,
        })

    trace = bool(int(os.environ.get("GAU_TRACE", "0")))
    res = run_bass_kernel_spmd(nc, in_maps, list(range(8)), trace=trace)
    kernel.last_exec_time_ns = res.exec_time_ns
    kernel.last_profile = res.profile_json

    full = np.empty((B, T, H), dtype=np.float32)
    for c in range(8):
        b, h = divmod(c, 2)
        full[b, h * TO:(h + 1) * TO] = res.results[c]["out"]
    return full


# revision 39
# speedup vs baseline: 1.0008x; 1.0008x over previous
"""GAU (relu^2 gated attention unit) kernel for 8 TRN2 NeuronCores.

Problem: B=4, T=2048, H=768 dense GAU block:
  x = LN(hidden); U,Q,K,V = silu(x @ W*^T + b*)
  sim = Q K^T / sqrt(T); A = relu(sim)^2; out = (U * (A V)) @ Wo^T + bo

Sharding: 8 cores = 4 batches x 2 sequence halves. Each core owns 1024
query rows; it recomputes K/V for its batch's full 2048 rows (cheaper
than cross-core collectives at this size). Inputs are passed per-core
with the OWN half first so all cores run the identical program
(relu^2 attention is permutation-invariant along the key axis).

Host-side weight preprocessing (weight-only algebra, no activations):
  - ln_gamma folded into W{u,q,k,v}; ln_beta+bias folded into per-proj bias
  - 1/sqrt(T)^2 = 1/T folded into Wo (relu(s*x)^2 = s^2 relu(x)^2)

On-device layouts (all matmuls fp32r = TF32, 1 cyc/row on the PE):
  - x^T [H, T] with H on partitions (6 k-subtiles) -> LN stats via
    ones-vector matmuls; normalize with per-column r/p broadcast tiles.
  - Q^T/K^T/U^T produced as [d, t] (silu+bias = one ACT op, bias on
    partitions); V natural [t, d] (lhsT of the AV matmul).
  - sim^T [j, i] = (K^T[:,j-tile]).T @ Q^T, so A^T[j, i] feeds the AV
    matmul directly: AV^T[d, i] = (V[j, d-tile]).T @ A^T. No transposes.
  - gating G = U^T * AV^T written in place into U^T; final projection
    contracts d with Wo^T [d, h] -> natural [t, h] output rows.
"""

import os
import numpy as np

import concourse.tile as tile
from concourse import bacc, mybir
from concourse.bass_utils import run_bass_kernel_spmd

F32 = mybir.dt.float32
F32R = mybir.dt.float32r

P = 128
H = 768
KS = H // P            # 6 k-subtiles over the H contraction
B = 4
T = 2048
TO = 1024              # own query rows per core
DT = H // P            # 6 d-tiles
CA = 512               # phase-A chunk (LN + Q/U)
NCA = T // CA          # 4
CC = 256               # phase-C chunk (K/V projections)
NCC = T // CC          # 8
NJ = T // P            # 16 key tiles
IH = 512               # i-half size for attention PSUM capacity
NIH = TO // IH         # 2
EPS = 1e-5

AF = mybir.ActivationFunctionType
ALU = mybir.AluOpType

_prog_cache = {}


def _build_program():
    nc = bacc.Bacc()
    xT = nc.declare_dram_parameter("xT", [H, T], F32R, isOutput=False)
    wq = nc.declare_dram_parameter("wq", [H, H], F32R, isOutput=False)
    wu = nc.declare_dram_parameter("wu", [H, H], F32R, isOutput=False)
    wk = nc.declare_dram_parameter("wk", [H, H], F32R, isOutput=False)
    wv = nc.declare_dram_parameter("wv", [H, H], F32R, isOutput=False)
    wo = nc.declare_dram_parameter("wo", [H, H], F32R, isOutput=False)
    bq = nc.declare_dram_parameter("bq", [H], F32, isOutput=False)
    bu = nc.declare_dram_parameter("bu", [H], F32, isOutput=False)
    bk = nc.declare_dram_parameter("bk", [H], F32, isOutput=False)
    bv = nc.declare_dram_parameter("bv", [H], F32, isOutput=False)
    bo = nc.declare_dram_parameter("bo", [H], F32, isOutput=False)
    out = nc.declare_dram_parameter("out", [TO, H], F32, isOutput=True)

    with tile.TileContext(nc) as tc:
        _emit(nc, tc, xT, (wq, wu, wk, wv, wo), (bq, bu, bk, bv, bo), out)
    nc.finalize()
    return nc


def _emit(nc, tc, xT, ws, bs, out):
    wq_d, wu_d, wk_d, wv_d, wo_d = ws
    bq_d, bu_d, bk_d, bv_d, bo_d = bs

    from contextlib import ExitStack

    with ExitStack() as ctx:
        const = ctx.enter_context(tc.tile_pool(name="const", bufs=1))
        big = ctx.enter_context(tc.tile_pool(name="big", bufs=1))
        wpool = ctx.enter_context(tc.tile_pool(name="wpool", bufs=2))
        dram = ctx.enter_context(tc.tile_pool(name="dram", bufs=1, space="DRAM"))

        # ---- constants (memsets only; bias DMAs are issued later so they
        # don't queue ahead of the critical first x-chunk loads) ----
        ones_f = const.tile([P, 1], F32, name="ones_f")
        nc.vector.memset(ones_f, 1.0)
        ones = const.tile([P, 1], F32R, name="ones")
        nc.vector.tensor_copy(ones, ones_f)
        ones_rf = const.tile([1, P], F32, name="ones_rf")
        nc.vector.memset(ones_rf, 1.0)
        ones_r = const.tile([1, P], F32R, name="ones_r")
        nc.vector.tensor_copy(ones_r, ones_rf)

        def load_biases():
            # per-partition biases for d-on-partition layouts: [128, 6]
            bq_t = const.tile([P, DT], F32, name="bq_t")
            nc.sync.dma_start(
                out=bq_t, in_=bq_d[:].rearrange("(dt p) -> p dt", p=P))
            bu_t = const.tile([P, DT], F32, name="bu_t")
            nc.sync.dma_start(
                out=bu_t, in_=bu_d[:].rearrange("(dt p) -> p dt", p=P))
            bk_t = const.tile([P, DT], F32, name="bk_t")
            nc.sync.dma_start(
                out=bk_t, in_=bk_d[:].rearrange("(dt p) -> p dt", p=P))
            # free-dim (replicated) biases for t-on-partition layouts
            bv_rep = const.tile([P, H], F32, name="bv_rep")
            nc.gpsimd.dma_start(
                out=bv_rep, in_=bv_d[:][None, :].to_broadcast([P, H]))
            bo_rep = const.tile([P, H], F32, name="bo_rep")
            nc.gpsimd.dma_start(
                out=bo_rep, in_=bo_d[:][None, :].to_broadcast([P, H]))
            return bq_t, bu_t, bk_t, bv_rep, bo_rep

        # ---- persistent activations (KT/VN created later, after phase A
        # pools release, so their SBUF space materializes late) ----
        QT = big.tile([P, DT, TO], F32R, name="QT")    # Q^T [d, t_own]
        UT = big.tile([P, DT, TO], F32R, name="UT")    # U^T; becomes G in place
        xhat_ch_dram = [
            dram.tile([P, KS, CA], F32R, name=f"xhat{c}") for c in range(NCA)]

        def load_w(pool, src_d, name):
            w = pool.tile([P, KS, H], F32R, tag="w", name=name)
            nc.sync.dma_start(
                out=w, in_=src_d[:].rearrange("(ks p) d -> p ks d", p=P))
            return w

        # =========== Phase A: LN stats + normalize (A0), Q/U (A1) ==========
        # ACT discipline: phase A uses only Sqrt / Square / Copy on the
        # scalar engine. Square and Copy are in every activation table set,
        # so the table never thrashes between the sqrt and silu sets. The
        # silu+bias for Q/U is applied as a batched pass at the end.
        pctx = ctx.enter_context(ExitStack())
        ps_pr = pctx.enter_context(
            tc.tile_pool(name="ps_pr", bufs=4, space="PSUM"))
        xs_pool = ctx.enter_context(tc.tile_pool(name="xs", bufs=2))
        xs_tiles = []
        wpA_ctx = ctx.enter_context(ExitStack())
        wpA = wpA_ctx.enter_context(tc.tile_pool(name="wpA", bufs=2))
        with ExitStack() as actx:
            xh = actx.enter_context(tc.tile_pool(name="xh", bufs=4))
            st = actx.enter_context(tc.tile_pool(name="st", bufs=2))
            ps_st = actx.enter_context(
                tc.tile_pool(name="ps_st", bufs=1, space="PSUM"))
            ps_bc = actx.enter_context(
                tc.tile_pool(name="ps_bc", bufs=1, space="PSUM"))

            # DMA issue order = need order: x chunks gate all early PE work;
            # each weight lands just before its first matmul.
            xT_r = xT[:].rearrange("(ks p) t -> p ks t", p=P)
            xh_ch = [xh.tile([P, KS, CA], F32R, tag="xh", bufs=4,
                             name=f"xc{ch}") for ch in range(NCA)]

            def load_xc(ch):
                for k in range(KS):
                    nc.sync.dma_start(
                        out=xh_ch[ch][:, k, :],
                        in_=xT_r[:, k, ch * CA:(ch + 1) * CA])

            load_xc(0)
            wq_t = load_w(wpA, wq_d, "wq_t")
            load_xc(1)
            wu_t = load_w(wpA, wu_d, "wu_t")
            load_xc(2)
            load_xc(3)
            bq_t, bu_t, bk_t, bv_rep, bo_rep = load_biases()
            wk_t = load_w(wpool, wk_d, "wk_t")
            wv_t = load_w(wpool, wv_d, "wv_t")

            # ---- A0: stats + normalize + spill ----
            for ch in range(NCA):
                sl = slice(ch * CA, (ch + 1) * CA)
                xc = xh_ch[ch]
                ps_s = ps_st.tile([1, CA], F32, tag="s", name=f"ps_s{ch}")
                ps_q = ps_st.tile([1, CA], F32, tag="ss", name=f"ps_q{ch}")
                for k in range(KS):
                    nc.tensor.matmul(ps_s, ones, xc[:, k, :],
                                     start=(k == 0), stop=(k == KS - 1))
                for k in range(KS):
                    x2 = st.tile([P, CA], F32R, tag="x2", name=f"x2_{ch}_{k}")
                    nc.scalar.activation(
                        out=x2, in_=xc[:, k, :], func=AF.Square)
                    nc.tensor.matmul(ps_q, ones, x2,
                                     start=(k == 0), stop=(k == KS - 1))
                s1 = st.tile([1, CA], F32, tag="s1", name=f"s1_{ch}", bufs=1)
                nc.vector.tensor_copy(s1, ps_s)
                q1 = st.tile([1, CA], F32, tag="q1", name=f"q1_{ch}", bufs=1)
                nc.vector.tensor_copy(q1, ps_q)
                # var = ss/H - (s/H)^2 ; r = rsqrt(var+eps); p = -(s/H)*r
                t1 = st.tile([1, CA], F32, tag="t1", name=f"t1_{ch}", bufs=1)
                nc.vector.scalar_tensor_tensor(
                    t1, s1, 1.0 / (H * H), s1, ALU.mult, ALU.mult)
                nc.vector.scalar_tensor_tensor(
                    q1, q1, 1.0 / H, t1, ALU.mult, ALU.subtract)
                nc.vector.tensor_scalar_add(q1, q1, EPS)
                nc.vector.reciprocal(t1, q1)
                r1 = st.tile([1, CA], F32R, tag="r1", name=f"r1_{ch}", bufs=1)
                nc.scalar.activation(out=r1, in_=t1, func=AF.Sqrt)
                p1 = st.tile([1, CA], F32R, tag="p1", name=f"p1_{ch}", bufs=1)
                nc.vector.scalar_tensor_tensor(
                    p1, s1, -1.0 / H, r1, ALU.mult, ALU.mult)
                # replicate r/p across partitions with a K=1 matmul
                rr = ps_bc.tile([P, CA], F32, tag="rr", name=f"rr{ch}")
                nc.tensor.matmul(rr, ones_r, r1, start=True, stop=True)
                pp = ps_bc.tile([P, CA], F32, tag="pp", name=f"pp{ch}")
                nc.tensor.matmul(pp, ones_r, p1, start=True, stop=True)
                for k in range(KS):
                    nc.vector.tensor_mul(xc[:, k, :], xc[:, k, :], rr)
                    nc.vector.tensor_add(xc[:, k, :], xc[:, k, :], pp)
                    nc.sync.dma_start(
                        out=xhat_ch_dram[ch][:, k, :], in_=xc[:, k, :])
                # pre-issue the phase-C reload DMAs for this chunk now, so
                # they sit ahead of later spills in the DMA queues
                for half in range(2):
                    xs = xs_pool.tile([P, KS, CC], F32R, tag="xs",
                                      name=f"xs{2 * ch + half}")
                    nc.sync.dma_start(
                        out=xs,
                        in_=xhat_ch_dram[ch][:, :, half * CC:(half + 1) * CC])
                    xs_tiles.append(xs)

            # ---- A1: Q/U projections from the own half. Copy-evict, then
            # silu+bias per d-tile as soon as both chunks are in (keeps the
            # sqrt/silu table sets from thrashing AND spreads the silu work
            # over A1's ACT slack instead of phase C's) ----
            def qu_mm(w_t, dst, ch, dt):
                sl = slice(ch * CA, (ch + 1) * CA)
                psum = ps_pr.tile([P, CA], F32, tag="pp512",
                                  name=f"pj{ch}_{dt}")
                for k in range(KS):
                    nc.tensor.matmul(
                        psum, w_t[:, k, dt * P:(dt + 1) * P],
                        xh_ch[ch][:, k, :],
                        start=(k == 0), stop=(k == KS - 1))
                nc.scalar.activation(out=dst[:, dt, sl], in_=psum, func=AF.Copy)

            for w_t, b_t, dst in ((wq_t, bq_t, QT), (wu_t, bu_t, UT)):
                for dt in range(DT):
                    qu_mm(w_t, dst, 0, dt)
                    qu_mm(w_t, dst, 1, dt)
                    nc.scalar.activation(
                        out=dst[:, dt, :], in_=dst[:, dt, :], func=AF.Silu,
                        bias=b_t[:, dt:dt + 1])
        wpA_ctx.close()

        # ========= Phase C: K/V projections (stream x^ back) =========
        kv = ctx.enter_context(tc.tile_pool(name="kv", bufs=1))
        KT = kv.tile([P, DT, T], F32R, name="KT")      # K^T [d, t_all]
        VN = kv.tile([P, NJ, H], F32R, name="VN")      # V natural [t_all, d]
        if True:
            for ch in range(NCC):
                sl = slice(ch * CC, (ch + 1) * CC)
                xs = xs_tiles[ch]
                # K^T [d, t]
                for dt in range(DT):
                    psum = ps_pr.tile([P, CC], F32, tag="pp512",
                                      name=f"pk{ch}_{dt}")
                    for k in range(KS):
                        nc.tensor.matmul(
                            psum, wk_t[:, k, dt * P:(dt + 1) * P], xs[:, k, :],
                            start=(k == 0), stop=(k == KS - 1))
                    nc.scalar.activation(
                        out=KT[:, dt, sl], in_=psum, func=AF.Silu,
                        bias=bk_t[:, dt:dt + 1])
                # V natural [t, d] (bias varies along free dim -> TT add + silu)
                for ts in range(CC // P):
                    tj = ch * (CC // P) + ts  # global t-subtile
                    for nh in range(2):
                        hsl = slice(nh * 384, (nh + 1) * 384)
                        psum = ps_pr.tile([P, 384], F32, tag="pp512",
                                          name=f"pv{ch}_{ts}_{nh}")
                        for k in range(KS):
                            nc.tensor.matmul(
                                psum, xs[:, k, ts * P:(ts + 1) * P],
                                wv_t[:, k, hsl],
                                start=(k == 0), stop=(k == KS - 1))
                        nc.vector.tensor_add(VN[:, tj, hsl], psum, bv_rep[:, hsl])
                    nc.scalar.activation(
                        out=VN[:, tj, :], in_=VN[:, tj, :], func=AF.Silu)

        pctx.close()

        # ================= Attention: sim^T -> relu^2 -> AV =================
        wo_t = load_w(wpool, wo_d, "wo_t")
        with ExitStack() as bctx:
            at_pool = bctx.enter_context(tc.tile_pool(name="at", bufs=2))
            ob_pool = bctx.enter_context(tc.tile_pool(name="ob", bufs=3))
            ps_sim = bctx.enter_context(
                tc.tile_pool(name="ps_sim", bufs=2, space="PSUM"))
            ps_av = bctx.enter_context(
                tc.tile_pool(name="ps_av", bufs=1, space="PSUM"))

            def out_proj(ih):
                # output projection for one i-half; psum reuses av tags so it
                # overlaps the other half's attention without extra banks
                for tt in range(ih * (IH // P), (ih + 1) * (IH // P)):
                    for nh in range(2):
                        hsl = slice(nh * 384, (nh + 1) * 384)
                        psum = ps_av.tile([P, IH], F32, tag=f"av{nh}",
                                          name=f"po{tt}_{nh}")[:, :384]
                        for dt in range(DT):
                            nc.tensor.matmul(
                                psum, UT[:, dt, tt * P:(tt + 1) * P],
                                wo_t[:, dt, hsl],
                                start=(dt == 0), stop=(dt == DT - 1))
                        osb = ob_pool.tile([P, 384], F32, tag="osb",
                                           name=f"osb{tt}_{nh}")
                        nc.vector.tensor_add(osb, psum, bo_rep[:, hsl])
                        nc.sync.dma_start(
                            out=out[:][tt * P:(tt + 1) * P, hsl], in_=osb)

            for ih in range(NIH):
                isl = slice(ih * IH, (ih + 1) * IH)
                ps_avs = [ps_av.tile([P, IH], F32, tag=f"av{dt}",
                                     name=f"av{ih}_{dt}") for dt in range(DT)]
                for jt in range(NJ):
                    psum = ps_sim.tile([P, IH], F32, tag="sim", name=f"sim{ih}_{jt}")
                    for k in range(KS):
                        nc.tensor.matmul(
                            psum, KT[:, k, jt * P:(jt + 1) * P], QT[:, k, isl],
                            start=(k == 0), stop=(k == KS - 1))
                    at = at_pool.tile([P, IH], F32R, tag="at", name=f"at{ih}_{jt}")
                    nc.scalar.activation(out=at, in_=psum, func=AF.Relu)
                    nc.vector.tensor_mul(at, at, at)
                    for dt in range(DT):
                        nc.tensor.matmul(
                            ps_avs[dt], VN[:, jt, dt * P:(dt + 1) * P], at,
                            start=(jt == 0), stop=(jt == NJ - 1))
                # gating: G = U^T * AV^T, in place into UT
                for dt in range(DT):
                    nc.vector.tensor_mul(
                        UT[:, dt, isl], ps_avs[dt], UT[:, dt, isl])
                out_proj(ih)


def kernel(**inputs):
    hs = np.ascontiguousarray(np.asarray(inputs["hidden_states"], dtype=np.float32))
    gamma = np.asarray(inputs["ln_gamma"], dtype=np.float32)
    beta = np.asarray(inputs["ln_beta"], dtype=np.float32)

    def prep_w(w):
        w = np.asarray(w, dtype=np.float32)
        return np.ascontiguousarray((w * gamma[None, :]).T)

    def prep_b(w, b):
        w = np.asarray(w, dtype=np.float32)
        b = np.asarray(b, dtype=np.float32)
        return (w @ beta + b).astype(np.float32)

    wq = prep_w(inputs["Wq_w"]);  bq = prep_b(inputs["Wq_w"], inputs["Wq_b"])
    wu = prep_w(inputs["Wu_w"]);  bu = prep_b(inputs["Wu_w"], inputs["Wu_b"])
    wk = prep_w(inputs["Wk_w"]);  bk = prep_b(inputs["Wk_w"], inputs["Wk_b"])
    wv = prep_w(inputs["Wv_w"]);  bv = prep_b(inputs["Wv_w"], inputs["Wv_b"])
    wo = np.ascontiguousarray(np.asarray(inputs["Wo_w"], np.float32).T) / np.float32(T)
    bo = np.asarray(inputs["Wo_b"], dtype=np.float32)

    if "nc" not in _prog_cache:
        _prog_cache["nc"] = _build_program()
    nc = _prog_cache["nc"]

    in_maps = []
    for c in range(8):
        b, h = divmod(c, 2)
        own = hs[b, h * TO:(h + 1) * TO]
        oth = hs[b, (1 - h) * TO:(2 - h) * TO]
        xT = np.ascontiguousarray(np.concatenate([own, oth], axis=0).T)
        in_maps.append({
            "xT": xT, "wq": wq, "wu": wu, "wk": wk, "wv": wv, "wo": wo,
            "bq": bq, "bu": bu, "bk": bk, "bv": bv, "bo": bo,
        })

    trace = bool(int(os.environ.get("GAU_TRACE", "0")))
    res = run_bass_kernel_spmd(nc, in_maps, list(range(8)), trace=trace)
    kernel.last_exec_time_ns = res.exec_time_ns
    kernel.last_profile = res.profile_json

    full = np.empty((B, T, H), dtype=np.float32)
    for c in range(8):
        b, h = divmod(c, 2)
        full[b, h * TO:(h + 1) * TO] = res.results[c]["out"]
    return full
